# revision 46
# baseline (speedup 1.0000x reference)
# Trainium2 Bass kernel for nn_Graph_AutoEncoder (BiLSTM encoder + GRU decoder).
#
# Sharding: decoder rows i in [256c, 256c+256) per core c. Each core encodes the
# 512 batch rows j = 2i, 2i+1 its decoder slice needs (LSTM1 both dirs at B=512,
# LSTM2 one direction). Cores 4-7 need the *backward* LSTM2 direction; they get
# time-reversed edge sequences and f/b-swapped LSTM1 weights via their input map,
# so the compiled program is identical on all 8 cores (SPMD, no collectives).
#
# The embedding table is sharded by the rows each core actually needs: the host
# gathers the (2 x 256) rows per core and ships the per-core mean, transposed
# (64 x 256) -- 64KB instead of the 12.8MB full table per core.
#
# Layout: feature-on-partition. Gates are computed as W @ h matmuls into PSUM
# (lhsT = W^T with K on partitions), biases folded either into x-projection
# matmuls (rhs = [x_t; ones]) or into ScalarE activation bias operands.
#
# Host<->device traffic over the axon tunnel is the wall-clock bottleneck
# (~47MB/s upload, ~6-15ms per extra input array, ~0.1s fixed output-fetch
# RTT; device exec is only ~10ms). Hence: ONE flat uint8 input per core
# holding fp8-e4m3 matmul weights (dequantized on device to bf16; validated
# ~2.4e-3 rel err vs the 2e-2 budget), fp8 edge sequences, bf16 small
# tensors and f32 ScalarE bias operands via bitcast views; bf16 compute
# states; bf16 output; a once-built cached jitted shard_map executor; and a
# single blocking np.asarray fetch (no separate block_until_ready round-trip).
import numpy as np

import concourse.bass as bass
import concourse.mybir as mybir
import concourse.tile as tile
from concourse import bacc
from concourse import bass2jax

F32 = mybir.dt.float32
BF = mybir.dt.bfloat16
F8 = mybir.dt.float8e4
U8 = mybir.dt.uint8
I32 = mybir.dt.int32
SIG = mybir.ActivationFunctionType.Sigmoid
TANH = mybir.ActivationFunctionType.Tanh
MUL = mybir.AluOpType.mult
ADD = mybir.AluOpType.add
SUB = mybir.AluOpType.subtract

T = 200
NC = 8
BE = 512  # encoder batch per core
BD = 256  # decoder batch per core

# wpack column offsets (128-partition matmul weights; shipped fp8, used bf16)
W_L1W = 0          # (128, 1024) l1_whhT both dirs
W_L2WIH = 1024     # (128, 1024)
W_L2WHH = 2048     # (128, 512)
W_FC1W = 2560      # (128, 512)
W_FC2W = 3072      # (128, 128)
W_G1WHH = 3200     # (128, 384)
W_G2WX = 3584      # (128, 150)
WCOLS = 3734

# bpack column offsets (f32 ScalarE activation-bias operands)
B_L2B = 0          # (128, 4)
B_FC1B = 4         # (128, 2)
B_DECB = 6         # (128, 1)
B_FC2B = 7         # (64, 1)
B_G2BN = 8         # (50, 1)
BCOLS = 9

# mpack column offsets (64-partition bf16; rows 0:50 for the 50-dim ones)
M_NMEMB = 0        # (64, 256) node embedding mean, transposed
M_G2WHH = 256      # (50, 150)
M_DECW = 406       # (50, 1)
MCOLS = 407

# spack column offsets (2-partition bf16; rows 0:1 for the 1-dim ones)
S_G1X = 0          # (2, 384)
S_DECI = 384       # (2, 256) [res0; ones]
S_ONES = 640       # (1, 512)
S_G1BHHN = 1152    # (1, 128)
S_G2BRZ = 1280     # (1, 100)
S_G2BHHN = 1380    # (1, 50)
SCOLS = 1430

# everything ships in one flat uint8 array per core; byte offsets:
AUX_W = 0                               # wpack fp8
AUX_EDGE = AUX_W + 128 * WCOLS          # edge fp8 (200, 512)
AUX_L1X = AUX_EDGE + T * BE             # l1 [wih; bias] fp8 (2, 1024)
AUX_M = AUX_L1X + 2 * 1024              # mpack bf16
AUX_S = AUX_M + 64 * MCOLS * 2          # spack bf16
AUX_B = AUX_S + 2 * SCOLS * 2           # bpack f32
NB = AUX_B + 128 * BCOLS * 4
assert AUX_M % 2 == 0 and AUX_S % 2 == 0 and AUX_B % 4 == 0

_CACHE = {}


def _build_program():
    nc = bacc.Bacc("TRN2", target_bir_lowering=False, debug=False, num_devices=NC)

    def din(name, shape, d=F32):
        return nc.dram_tensor(name, shape, d, kind="ExternalInput").ap()

    aux = din("aux", [1, NB], U8)

    def aview(off, dt, p, c):
        esz = {F8: 1, BF: 2, F32: 4}[dt]
        return aux[0:1, off : off + p * c * esz].bitcast(dt).rearrange(
            "one (p c) -> (one p) c", p=p)

    def erow_src(s):
        off = AUX_EDGE + BE * s
        return aux[0:1, off : off + BE].bitcast(F8)

    out_staged = nc.dram_tensor("out_staged", [25, 8 * BD], BF, kind="ExternalOutput").ap()
    h1_buf = nc.dram_tensor("h1_buf", [2, T, 128, BE], BF).ap()

    with tile.TileContext(nc) as tc:
        with (
            tc.tile_pool(name="wpool", bufs=1) as wp,
            tc.tile_pool(name="spool", bufs=1) as sp,
        ):
            # ---- persistent packed weights / small tensors ----
            wt = wp.tile([128, WCOLS], BF)
            with tc.tile_pool(name="wstage", bufs=1) as stg:
                wt8 = stg.tile([128, WCOLS], F8)
                nc.sync.dma_start(out=wt8[:], in_=aview(AUX_W, F8, 128, WCOLS))
                nc.vector.tensor_copy(out=wt[:], in_=wt8[:])  # fp8 -> bf16
            mt = wp.tile([64, MCOLS], BF)
            nc.sync.dma_start(out=mt[:], in_=aview(AUX_M, BF, 64, MCOLS))
            st = wp.tile([2, SCOLS], BF)
            nc.sync.dma_start(out=st[:], in_=aview(AUX_S, BF, 2, SCOLS))
            bt = wp.tile([128, BCOLS], F32)
            nc.sync.dma_start(out=bt[:], in_=aview(AUX_B, F32, 128, BCOLS))
            l1x = wp.tile([2, 1024], F8)
            nc.sync.dma_start(out=l1x[:], in_=aview(AUX_L1X, F8, 2, 1024))

            l1w = wt[:, W_L1W : W_L1W + 1024]
            l2wih = wt[:, W_L2WIH : W_L2WIH + 1024]
            l2whh = wt[:, W_L2WHH : W_L2WHH + 512]
            fc1w = wt[:, W_FC1W : W_FC1W + 512]
            fc2w = wt[:, W_FC2W : W_FC2W + 128]
            g1whh = wt[:, W_G1WHH : W_G1WHH + 384]
            g2wx = wt[:, W_G2WX : W_G2WX + 150]

            l2b = bt[:, B_L2B : B_L2B + 4]
            fc1b = bt[:, B_FC1B : B_FC1B + 2]
            decb = bt[:, B_DECB : B_DECB + 1]
            fc2b = bt[0:64, B_FC2B : B_FC2B + 1]
            g2bn = bt[0:50, B_G2BN : B_G2BN + 1]

            g2whh = mt[0:50, M_G2WHH : M_G2WHH + 150]
            decw = mt[0:50, M_DECW : M_DECW + 1]

            g1x = st[:, S_G1X : S_G1X + 384]
            deci = st[:, S_DECI : S_DECI + 256]
            ones256 = st[0:1, S_ONES : S_ONES + 256]
            g1bhhn = st[0:1, S_G1BHHN : S_G1BHHN + 128]
            g2brz = st[0:1, S_G2BRZ : S_G2BRZ + 100]
            g2bhhn = st[0:1, S_G2BHHN : S_G2BHHN + 50]

            # ---- edge rings: persistent [x_s; ones] tiles; DMA refills row 0
            # each step (PE rhs base partition must be 32-aligned, and SBUF
            # pools charge free-dim bytes on every partition, so small 2-row
            # tiles with an immortal ones-row are the cheapest layout).
            ring_f = [wp.tile([2, BE], F8, name=f"erf{i}") for i in range(8)]
            ring_b = [wp.tile([2, BE], F8, name=f"erb{i}") for i in range(8)]
            for rt in ring_f + ring_b:
                nc.vector.memset(rt[:], 1.0)

            # ---- persistent state ----
            hn1 = sp.tile([128, BE], BF)
            hn2cap = sp.tile([128, BE], BF)
            hinit = sp.tile([128, BD], F32)
            hb = sp.tile([128, BD], BF)      # bf16 shadow of hinit (matmul rhs)
            h2g = sp.tile([50, BD], F32)
            h2gb = sp.tile([50, BD], BF)     # bf16 shadow of h2g
            res = sp.tile([2, BD], BF)

            # ================= LSTM1 (both dirs, B=512) =================
            with (
                tc.tile_pool(name="l1hring", bufs=4) as hp,
                tc.tile_pool(name="l1work", bufs=3) as kp,
                tc.tile_pool(name="l1state", bufs=1) as lsp,
                tc.tile_pool(name="l1psum", bufs=1, space="PSUM") as pp,
            ):
                c1 = lsp.tile([128, 1024], F32)
                nc.vector.memset(c1[:], 0.0)
                h_prev = hp.tile([128, 1024], BF, tag="h1o")
                nc.vector.memset(h_prev[:], 0.0)
                for s in range(T):
                    erf = ring_f[s % 8]
                    nc.sync.dma_start(out=erf[0:1, :], in_=erow_src(s))
                    erb = ring_b[s % 8]
                    nc.sync.dma_start(out=erb[0:1, :], in_=erow_src(T - 1 - s))
                    sigp = pp.tile([128, 3072], F32, tag="sigp", space="PSUM")
                    gp = pp.tile([128, 1024], F32, tag="gp", space="PSUM")
                    for d in (0, 1):
                        hs_d = h_prev[:, 512 * d : 512 * d + 512]
                        er_d = erf[:] if d == 0 else erb[:]
                        for gi, g in ((0, 0), (1, 1), (2, 3), (3, 2)):
                            if g == 2:  # tanh gate
                                dst = gp[:, 512 * d : 512 * d + 512]
                            else:
                                col = (0, 1, None, 2)[g]
                                dst = sigp[:, 1024 * col + 512 * d : 1024 * col + 512 * d + 512]
                            nc.tensor.matmul(
                                dst, lhsT=l1w[:, 512 * d + 128 * g : 512 * d + 128 * g + 128],
                                rhs=hs_d, start=True, stop=False)
                            nc.tensor.matmul(
                                dst, lhsT=l1x[:, 512 * d + 128 * g : 512 * d + 128 * g + 128],
                                rhs=er_d, start=False, stop=True)
                    sig_sb = kp.tile([128, 3072], F32, tag="sig_sb")
                    nc.scalar.activation(sig_sb[:], sigp[:], SIG)
                    g_sb = kp.tile([128, 1024], F32, tag="g_sb")
                    nc.scalar.activation(g_sb[:], gp[:], TANH)
                    t1 = kp.tile([128, 1024], F32, tag="t1")
                    nc.vector.tensor_tensor(out=t1[:], in0=sig_sb[:, 0:1024], in1=g_sb[:], op=MUL)
                    nc.vector.tensor_tensor(out=c1[:], in0=sig_sb[:, 1024:2048], in1=c1[:], op=MUL)
                    nc.vector.tensor_tensor(out=c1[:], in0=c1[:], in1=t1[:], op=ADD)
                    tc_sb = kp.tile([128, 1024], F32, tag="tc_sb")
                    nc.scalar.activation(tc_sb[:], c1[:], TANH)
                    h_cur = hp.tile([128, 1024], BF, tag="h1o")
                    nc.vector.tensor_tensor(out=h_cur[:], in0=sig_sb[:, 2048:3072], in1=tc_sb[:], op=MUL)
                    nc.sync.dma_start(out=h1_buf[0, s], in_=h_cur[:, 0:512])
                    nc.sync.dma_start(out=h1_buf[1, T - 1 - s], in_=h_cur[:, 512:1024])
                    h_prev = h_cur
                nc.vector.tensor_copy(out=hn1[:], in_=h_prev[:, 0:512])

            # ================= LSTM2 (one dir, B=512) =================
            with (
                tc.tile_pool(name="l2ring", bufs=8) as rp2,
                tc.tile_pool(name="l2work", bufs=3) as kp2,
                tc.tile_pool(name="l2state", bufs=1) as lsp2,
                tc.tile_pool(name="l2psum", bufs=2, space="PSUM") as pp2,
            ):
                c2 = lsp2.tile([128, 512], F32)
                nc.vector.memset(c2[:], 0.0)
                h2p = lsp2.tile([128, 512], BF)
                nc.vector.memset(h2p[:], 0.0)
                h2n = lsp2.tile([128, 512], BF)
                for s in range(T):
                    xf = rp2.tile([128, 512], BF, tag="xf")
                    nc.sync.dma_start(out=xf[:], in_=h1_buf[0, s])
                    xb = rp2.tile([128, 512], BF, tag="xb")
                    nc.sync.dma_start(out=xb[:], in_=h1_buf[1, s])
                    sp2t = pp2.tile([128, 1536], F32, tag="sp2", space="PSUM")
                    gp2 = pp2.tile([128, 512], F32, tag="gp2", space="PSUM")
                    for g, dst_info in ((0, (sp2t, 0)), (1, (sp2t, 512)), (3, (sp2t, 1024)), (2, (gp2, 0))):
                        dtile, off = dst_info
                        dst = dtile[:, off : off + 512]
                        nc.tensor.matmul(dst, lhsT=l2wih[:, 128 * g : 128 * g + 128], rhs=xf[:], start=True, stop=False)
                        nc.tensor.matmul(dst, lhsT=l2wih[:, 512 + 128 * g : 512 + 128 * g + 128], rhs=xb[:], start=False, stop=False)
                        nc.tensor.matmul(dst, lhsT=l2whh[:, 128 * g : 128 * g + 128], rhs=h2p[:], start=False, stop=True)
                    sb2 = kp2.tile([128, 1536], F32, tag="sb2")
                    nc.scalar.activation(sb2[:, 0:512], sp2t[:, 0:512], SIG, bias=l2b[:, 0:1])
                    nc.scalar.activation(sb2[:, 512:1024], sp2t[:, 512:1024], SIG, bias=l2b[:, 1:2])
                    nc.scalar.activation(sb2[:, 1024:1536], sp2t[:, 1024:1536], SIG, bias=l2b[:, 3:4])
                    g2sb = kp2.tile([128, 512], F32, tag="g2sb")
                    nc.scalar.activation(g2sb[:], gp2[:], TANH, bias=l2b[:, 2:3])
                    t2 = kp2.tile([128, 512], F32, tag="t2")
                    nc.vector.tensor_tensor(out=t2[:], in0=sb2[:, 0:512], in1=g2sb[:], op=MUL)
                    nc.vector.tensor_tensor(out=c2[:], in0=sb2[:, 512:1024], in1=c2[:], op=MUL)
                    nc.vector.tensor_tensor(out=c2[:], in0=c2[:], in1=t2[:], op=ADD)
                    tc2 = kp2.tile([128, 512], F32, tag="tc2")
                    nc.scalar.activation(tc2[:], c2[:], TANH)
                    dst_h = hn2cap if s == T - 1 else (h2n if s % 2 == 0 else h2p)
                    nc.vector.tensor_tensor(out=dst_h[:], in0=sb2[:, 1024:1536], in1=tc2[:], op=MUL)
                    h2p, h2n = dst_h, (h2p if s % 2 == 0 else h2n)

            # ================= encoder tail =================
            with (
                tc.tile_pool(name="etwork", bufs=1) as ep,
                tc.tile_pool(name="etpsum", bufs=1, space="PSUM") as epp,
            ):
                hnsum = ep.tile([128, BE], BF)
                nc.vector.tensor_tensor(out=hnsum[:], in0=hn1[:], in1=hn2cap[:], op=ADD)
                X = ep.tile([128, 512], BF)
                hv = hnsum[:].rearrange("p (k two) -> p two k", two=2)
                nc.vector.tensor_copy(out=X[:, 0:256], in_=hv[:, 0, :])
                nc.vector.tensor_copy(out=X[:, 256:512], in_=hv[:, 1, :])
                fc1p = epp.tile([128, 512], F32, tag="fc1p", space="PSUM")
                for m in (0, 1):
                    dst = fc1p[:, 256 * m : 256 * m + 256]
                    nc.tensor.matmul(dst, lhsT=fc1w[:, 128 * m : 128 * m + 128], rhs=X[:, 0:256], start=True, stop=False)
                    nc.tensor.matmul(dst, lhsT=fc1w[:, 256 + 128 * m : 256 + 128 * m + 128], rhs=X[:, 256:512], start=False, stop=True)
                Y = ep.tile([128, 512], BF)
                nc.scalar.activation(Y[:, 0:256], fc1p[:, 0:256], SIG, bias=fc1b[:, 0:1])
                nc.scalar.activation(Y[:, 256:512], fc1p[:, 256:512], SIG, bias=fc1b[:, 1:2])
                fc2p = epp.tile([64, 256], F32, tag="fc2p", space="PSUM")
                nc.tensor.matmul(fc2p[:], lhsT=fc2w[:, 0:64], rhs=Y[:, 0:256], start=True, stop=False)
                nc.tensor.matmul(fc2p[:], lhsT=fc2w[:, 64:128], rhs=Y[:, 256:512], start=False, stop=True)
                nc.scalar.activation(hinit[0:64, :], fc2p[:], SIG, bias=fc2b)
                # node embedding mean: pre-gathered on host, just place it
                nc.vector.tensor_copy(out=hinit[64:128, :], in_=mt[:, M_NMEMB : M_NMEMB + 256])
                nc.vector.tensor_copy(out=hb[:], in_=hinit[:])

            # ================= decoder (B=256) =================
            with (
                tc.tile_pool(name="dwork", bufs=3) as dp_pool,
                tc.tile_pool(name="dpsum", bufs=2, space="PSUM") as dpp,
                tc.tile_pool(name="dpsum1", bufs=1, space="PSUM") as dpp1,
            ):
                nc.vector.memset(h2g[:], 0.0)
                nc.vector.memset(h2gb[:], 0.0)
                nc.vector.tensor_copy(out=res[:], in_=deci)
                for t in range(T):
                    g1p = dpp.tile([128, 1024], F32, tag="g1p", space="PSUM")
                    nc.tensor.matmul(g1p[:, 0:256], lhsT=g1whh[:, 0:128], rhs=hb[:], start=True, stop=False)
                    nc.tensor.matmul(g1p[:, 0:256], lhsT=g1x[:, 0:128], rhs=res[:], start=False, stop=True)
                    nc.tensor.matmul(g1p[:, 256:512], lhsT=g1whh[:, 128:256], rhs=hb[:], start=True, stop=False)
                    nc.tensor.matmul(g1p[:, 256:512], lhsT=g1x[:, 128:256], rhs=res[:], start=False, stop=True)
                    nc.tensor.matmul(g1p[:, 512:768], lhsT=g1x[:, 256:384], rhs=res[:], start=True, stop=True)
                    nc.tensor.matmul(g1p[:, 768:1024], lhsT=g1whh[:, 256:384], rhs=hb[:], start=True, stop=False)
                    nc.tensor.matmul(g1p[:, 768:1024], lhsT=g1bhhn, rhs=ones256, start=False, stop=True)
                    rz_sb = dp_pool.tile([128, 512], F32, tag="rz_sb")
                    nc.scalar.activation(rz_sb[:], g1p[:, 0:512], SIG)
                    tt = dp_pool.tile([128, 256], F32, tag="tt")
                    nc.vector.tensor_tensor(out=tt[:], in0=rz_sb[:, 0:256], in1=g1p[:, 768:1024], op=MUL)
                    nc.vector.tensor_tensor(out=tt[:], in0=tt[:], in1=g1p[:, 512:768], op=ADD)
                    n_sb = dp_pool.tile([128, 256], F32, tag="n_sb")
                    nc.scalar.activation(n_sb[:], tt[:], TANH)
                    dtl = dp_pool.tile([128, 256], F32, tag="dtl")
                    nc.gpsimd.tensor_tensor(out=dtl[:], in0=hinit[:], in1=n_sb[:], op=SUB)
                    nc.gpsimd.tensor_tensor(out=dtl[:], in0=rz_sb[:, 256:512], in1=dtl[:], op=MUL)
                    nc.gpsimd.tensor_tensor(out=hinit[:], in0=n_sb[:], in1=dtl[:], op=ADD)
                    nc.vector.tensor_copy(out=hb[:], in_=hinit[:])
                    # GRU2
                    g2p = dpp1.tile([50, 1024], F32, tag="g2p", space="PSUM")
                    nc.tensor.matmul(g2p[:, 0:256], lhsT=g2wx[:, 0:50], rhs=hb[:], start=True, stop=False)
                    nc.tensor.matmul(g2p[:, 0:256], lhsT=g2whh[:, 0:50], rhs=h2gb[:], start=False, stop=False)
                    nc.tensor.matmul(g2p[:, 0:256], lhsT=g2brz[:, 0:50], rhs=ones256, start=False, stop=True)
                    nc.tensor.matmul(g2p[:, 256:512], lhsT=g2wx[:, 50:100], rhs=hb[:], start=True, stop=False)
                    nc.tensor.matmul(g2p[:, 256:512], lhsT=g2whh[:, 50:100], rhs=h2gb[:], start=False, stop=False)
                    nc.tensor.matmul(g2p[:, 256:512], lhsT=g2brz[:, 50:100], rhs=ones256, start=False, stop=True)
                    nc.tensor.matmul(g2p[:, 512:768], lhsT=g2wx[:, 100:150], rhs=hb[:], start=True, stop=True)
                    nc.tensor.matmul(g2p[:, 768:1024], lhsT=g2whh[:, 100:150], rhs=h2gb[:], start=True, stop=False)
                    nc.tensor.matmul(g2p[:, 768:1024], lhsT=g2bhhn, rhs=ones256, start=False, stop=True)
                    rz2 = dp_pool.tile([50, 512], F32, tag="rz2")
                    nc.scalar.activation(rz2[:], g2p[:, 0:512], SIG)
                    t2t = dp_pool.tile([50, 256], F32, tag="t2t")
                    nc.vector.tensor_tensor(out=t2t[:], in0=rz2[:, 0:256], in1=g2p[:, 768:1024], op=MUL)
                    nc.vector.tensor_tensor(out=t2t[:], in0=t2t[:], in1=g2p[:, 512:768], op=ADD)
                    n2 = dp_pool.tile([50, 256], F32, tag="n2")
                    nc.scalar.activation(n2[:], t2t[:], TANH, bias=g2bn)
                    d2 = dp_pool.tile([50, 256], F32, tag="d2")
                    nc.vector.tensor_tensor(out=d2[:], in0=h2g[:], in1=n2[:], op=SUB)
                    nc.vector.tensor_tensor(out=d2[:], in0=rz2[:, 256:512], in1=d2[:], op=MUL)
                    nc.vector.tensor_tensor(out=h2g[:], in0=n2[:], in1=d2[:], op=ADD)
                    nc.vector.tensor_copy(out=h2gb[:], in_=h2g[:])
                    # dec fc (single output row)
                    dcp = dpp.tile([1, 256], F32, tag="dcp", space="PSUM")
                    nc.tensor.matmul(dcp[:], lhsT=decw, rhs=h2gb[:], start=True, stop=True)
                    nc.scalar.activation(res[0:1, :], dcp[0:1, :], SIG, bias=decb[0:1, :])
                    k = t % 8
                    if k == 0:
                        oblk = dp_pool.tile([1, 8 * BD], BF, tag="oblk")
                    nc.scalar.activation(
                        oblk[0:1, BD * k : BD * k + BD], dcp[0:1, :], SIG,
                        bias=decb[0:1, :])
                    if k == 7:
                        nc.sync.dma_start(out=out_staged[t // 8], in_=oblk[:])

    nc.finalize()
    return nc


def _get_exec():
    if "exec" in _CACHE:
        return _CACHE["exec"]
    import jax
    from jax.sharding import Mesh, PartitionSpec
    from jax.experimental.shard_map import shard_map

    nc = _build_program()
    bass2jax.install_neuronx_cc_hook()
    partition_name = nc.partition_id_tensor.name if nc.partition_id_tensor else None
    in_names, out_names, out_avals = [], [], []
    for alloc in nc.m.functions[0].allocations:
        if not isinstance(alloc, mybir.MemoryLocationSet):
            continue
        name = alloc.memorylocations[0].name
        if alloc.kind == "ExternalInput":
            if name != partition_name:
                in_names.append(name)
        elif alloc.kind == "ExternalOutput":
            out_names.append(name)
            out_avals.append(jax.core.ShapedArray(
                tuple(alloc.tensor_shape), mybir.dt.np(alloc.dtype)))
    n_params = len(in_names)
    n_outs = len(out_avals)
    all_names = in_names + out_names
    donate = tuple(range(n_params, n_params + n_outs))

    def _body(*args):
        operands = list(args)
        if partition_name is not None:
            operands.append(bass2jax.partition_id_tensor())
        outs = bass2jax._bass_exec_p.bind(
            *operands, out_avals=tuple(out_avals),
            in_names=tuple(all_names if partition_name is None else all_names + [partition_name]),
            out_names=tuple(out_names), lowering_input_output_aliases=(),
            sim_require_finite=True, sim_require_nnan=True, nc=nc)
        return tuple(outs)

    devices = jax.devices()[:NC]
    mesh = Mesh(np.asarray(devices), ("core",))
    sharded = jax.jit(
        shard_map(_body, mesh=mesh,
                  in_specs=(PartitionSpec("core"),) * (n_params + n_outs),
                  out_specs=(PartitionSpec("core"),) * n_outs, check_rep=False),
        donate_argnums=donate, keep_unused=True)
    ex = dict(nc=nc, in_names=in_names, out_names=out_names,
              out_avals=out_avals, sharded=sharded, jax=jax)
    _CACHE["exec"] = ex
    return ex


_F8LUT = None


def _f8(a):
    """f32 -> fp8-e4m3, via a bf16 cast + 65536-entry LUT (ml_dtypes' direct
    cast runs at only ~100MB/s; this is ~4x faster and exact RNE from bf16)."""
    global _F8LUT
    import ml_dtypes
    if _F8LUT is None:
        all16 = np.arange(65536, dtype=np.uint16).view(ml_dtypes.bfloat16)
        with np.errstate(invalid="ignore"):  # inf/NaN bf16 codes are never indexed
            _F8LUT = all16.astype(np.float32).astype(ml_dtypes.float8_e4m3).view(np.uint8)
    idx = np.ascontiguousarray(a, np.float32).astype(ml_dtypes.bfloat16).view(np.uint16)
    return _F8LUT[idx].view(ml_dtypes.float8_e4m3)


def _prep_global_inputs(inputs):
    """Build the single concatenated (8, NB) flat uint8 input array."""
    import ml_dtypes
    BF_NP = ml_dtypes.bfloat16
    F8_NP = ml_dtypes.float8_e4m3

    def u8(a):
        return np.ascontiguousarray(a).reshape(-1).view(np.uint8)

    inp = {k: np.asarray(v) for k, v in inputs.items()}
    edge = np.ascontiguousarray(inp["edge_data"][:, :, 0]).astype(np.float32)  # (2048, 200)
    node = inp["node_data"]
    emb = np.asarray(inp["emb"], np.float32)
    aux_g = np.empty((NC, NB), np.uint8)  # every region below is fully written

    # ---- wpack fp8: two variants (cores 0-3 fwd LSTM2, 4-7 bwd); convert
    # each unique block to fp8 once and assemble variants from the pieces ----
    l1_8 = {d: np.asarray(_f8(inp[f"l1_whh_{d}"].T)) for d in "fb"}
    l2wih_8 = {d: np.asarray(_f8(np.ascontiguousarray(
        inp[f"l2_wih_{d}"].T.reshape(2, 128, 512).transpose(1, 0, 2)
    ).reshape(128, 1024))) for d in "fb"}
    l2whh_8 = {d: np.asarray(_f8(inp[f"l2_whh_{d}"].T)) for d in "fb"}
    shared = np.zeros((128, WCOLS - W_FC1W), np.float32)
    fc1wT = inp["fc1_w"].T.reshape(2, 128, 256)
    shared[:, 0:256] = fc1wT[0]
    shared[:, 256:512] = fc1wT[1]
    fc2wT = inp["fc2_w"].T.reshape(2, 128, 64)
    shared[:, 512:576] = fc2wT[0]
    shared[:, 576:640] = fc2wT[1]
    shared[:, 640:1024] = inp["g1_whh"].T
    shared[:, 1024:1174] = inp["g2_wih"].T
    shared_8 = np.asarray(_f8(shared))

    def wvariant(l1o, d2):
        w = np.empty((128, WCOLS), F8_NP)
        w[:, W_L1W : W_L1W + 512] = l1_8[l1o[0]]
        w[:, W_L1W + 512 : W_L1W + 1024] = l1_8[l1o[1]]
        w[:, W_L2WIH : W_L2WIH + 1024] = l2wih_8[d2]
        w[:, W_L2WHH : W_L2WHH + 512] = l2whh_8[d2]
        w[:, W_FC1W:] = shared_8
        return u8(w)

    w8 = {True: wvariant("fb", "f"), False: wvariant("bf", "b")}

    # ---- l1x fp8 variants ----
    lx = np.zeros((2, 2, 1024), np.float32)
    for v, order in ((0, "fb"), (1, "bf")):
        for d, nm in enumerate(order):
            lx[v, 0, 512 * d : 512 * d + 512] = inp[f"l1_wih_{nm}"][:, 0]
            lx[v, 1, 512 * d : 512 * d + 512] = inp[f"l1_b_{nm}"]
    lx8 = {True: u8(_f8(lx[0])), False: u8(_f8(lx[1]))}

    # ---- bpack f32 variants (l2b is direction-dependent) ----
    def bvariant(d2):
        b = np.zeros((128, BCOLS), np.float32)
        b[:, B_L2B : B_L2B + 4] = inp[f"l2_b_{d2}"].reshape(4, 128).T
        b[:, B_FC1B : B_FC1B + 2] = inp["fc1_b"].reshape(2, 128).T
        b[:, B_DECB] = inp["dec_b"][0]
        b[0:64, B_FC2B] = inp["fc2_b"]
        b[0:50, B_G2BN] = inp["g2_bih"][100:150]
        return u8(b)

    bb = {True: bvariant("f"), False: bvariant("b")}

    # ---- mpack bf16 (node_memb is per-core) ----
    node_mean = 0.5 * (emb[node[:, 0]] + emb[node[:, 1]])  # (2048, 64)
    m0 = np.zeros((64, MCOLS), np.float32)
    m0[0:50, M_G2WHH : M_G2WHH + 150] = inp["g2_whh"].T
    m0[0:50, M_DECW] = inp["dec_w"][0]

    # ---- spack bf16 (dec_init is per-core) ----
    g1_bias = np.concatenate(
        [(inp["g1_bih"] + inp["g1_bhh"])[0:256], inp["g1_bih"][256:384]])
    s0 = np.zeros((2, SCOLS), np.float32)
    s0[0, S_G1X : S_G1X + 384] = inp["g1_wih"][:, 0]
    s0[1, S_G1X : S_G1X + 384] = g1_bias
    s0[1, S_DECI : S_DECI + 256] = 1.0
    s0[0, S_ONES : S_ONES + 512] = 1.0
    s0[0, S_G1BHHN : S_G1BHHN + 128] = inp["g1_bhh"][256:384]
    s0[0, S_G2BRZ : S_G2BRZ + 100] = inp["g2_bih"][0:100] + inp["g2_bhh"][0:100]
    s0[0, S_G2BHHN : S_G2BHHN + 50] = inp["g2_bhh"][100:150]

    edge8 = np.asarray(_f8(edge))  # one bulk conversion, sliced per core below
    m0_u8 = u8(m0.astype(BF_NP))
    s0_u8 = u8(s0.astype(BF_NP))
    nmean_bf = node_mean.astype(BF_NP)

    def fill_core(c):
        fwd = c < 4
        a = aux_g[c]
        a[AUX_W : AUX_W + 128 * WCOLS] = w8[fwd]
        eT = edge8[512 * (c % 4) : 512 * (c % 4) + 512].T  # (200, 512) fp8
        if not fwd:
            eT = eT[::-1]
        a[AUX_EDGE : AUX_EDGE + T * BE] = u8(eT)
        a[AUX_L1X : AUX_L1X + 2048] = lx8[fwd]
        a[AUX_M : AUX_M + 64 * MCOLS * 2] = m0_u8
        mv = a[AUX_M : AUX_M + 64 * MCOLS * 2].view(BF_NP).reshape(64, MCOLS)
        mv[:, M_NMEMB : M_NMEMB + 256] = nmean_bf[BD * c : BD * c + BD].T
        a[AUX_S : AUX_S + 2 * SCOLS * 2] = s0_u8
        sv = a[AUX_S : AUX_S + 2 * SCOLS * 2].view(BF_NP).reshape(2, SCOLS)
        sv[0, S_DECI : S_DECI + 256] = edge[BD * c : BD * c + BD, -1].astype(BF_NP)
        a[AUX_B : AUX_B + 128 * BCOLS * 4] = bb[fwd]

    from concurrent.futures import ThreadPoolExecutor
    with ThreadPoolExecutor(NC) as pool:
        list(pool.map(fill_core, range(NC)))
    return {"aux": aux_g}


def run_device(inputs, time_parts=False):
    import time as _time
    ex = _get_exec()
    t0 = _time.time()
    glob = _prep_global_inputs(inputs)
    t1 = _time.time()
    args = [glob[name] for name in ex["in_names"]]

    def attempt(obufs):
        # The kernel writes every element of its outputs, so the donated
        # backing buffers need no particular contents: recycle the previous
        # call's device-resident outputs instead of uploading fresh zeros.
        if obufs is None:
            obufs = [np.zeros((NC * av.shape[0], *av.shape[1:]), av.dtype)
                     for av in ex["out_avals"]]
        out_arrs = ex["sharded"](*args, *obufs)
        # np.asarray blocks internally; an explicit block_until_ready first
        # would cost an extra ~80ms tunnel round-trip.
        host = {name: np.asarray(a)
                for name, a in zip(ex["out_names"], out_arrs)}
        return host, list(out_arrs)

    try:
        host, obufs = attempt(_CACHE.pop("obufs", None))
    except Exception:
        _time.sleep(3.0)  # transient tunnel/device hiccup: one retry
        host, obufs = attempt(None)
    _CACHE["obufs"] = obufs
    t2 = t3 = _time.time()
    staged = host["out_staged"].astype(np.float32).reshape(NC, 25, 8, BD)
    out = np.zeros((2048, T, 1), np.float32)
    for c in range(NC):
        out[256 * c : 256 * c + 256, :, 0] = staged[c].reshape(T, BD).T
    t4 = _time.time()
    if time_parts:
        print(f"  parts: prep {t1-t0:.3f} xfer+exec {t2-t1:.3f} "
              f"fetch {t3-t2:.3f} post {t4-t3:.3f}")
    return out, None


def kernel(**inputs) -> np.ndarray:
    out, _ = run_device(inputs)
    return out
